# revision 13
# baseline (speedup 1.0000x reference)
"""Trainium2 Bass kernel for nn_AssignAttention (hard-assignment MoE-routing attention).

Math (forward): for each (b, h, key-token s), the key token is hard-assigned to
group n* = argmax_n (q_bhn . k_bhs); output per group = sum of assigned v vectors
scaled by 1/(count+1), then projected.  The straight-through softmax terms cancel
in forward up to ~1e-7, so only the argmax routing matters.

Strategy:
 - Pure data-parallel over batch B=16 across 8 cores (2 batches/core), no collectives.
 - Host precomputes t[b,h,n,:] = Wk_h^T Wq_h query[b,n] so attention logits are
   attn[s, (h,n)] = key[b,s,:] . t[b,h,n,:]  -- one C-contraction against raw key.
 - Host lays out every device tensor partition-major ([128, ct, ...] with each
   partition's data contiguous in HBM): DMA descriptor count per transfer drops
   3x (128 x 6KB instead of 384 x 2KB), which cuts both the serial DIRECT2D
   trigger cost on the sequencers and per-descriptor queue overhead.
 - Attention logits use float32r matmuls (1 cyc/row, ~13-bit mantissa): measured
   argmax flip-induced error ~0.008 rel, well within tolerance. v/output paths in
   float32r/bf16.
 - Per 128-row s-subtile: argmax over each head's 64 logit columns (free-axis
   reduce_max + one broadcast is_equal -> bf16 one-hot), then PSUM-accumulate
   head-PAIR-packed o += aT_pair^T @ [v|1|v|1] (128-col bf16 lhsT enables fast
   weight loads; the ones column yields per-group counts).  The o-matmuls of
   subtile i are issued after subtile i+1's attn/v matmuls so the PE never
   head-of-line blocks on the DVE one-hot.  Epilogue scales by 1/(cnt+1)
   straight out of PSUM, pipelined per head-pair across DVE/PE/Act.
 - Startup latency: the NEFF boot sequence blocks every sequencer ~6-7us while
   the DMA queues sit idle, so the first chunk is only 128 tokens and the first
   transfers are split between the Activation and SP queues (the only HWDGE
   trigger engines) with per-ct tc pieces, putting subtile 0's inputs on SBUF
   as early as possible after boot.
"""
import sys

sys.path.insert(0, "/opt/trn_rl_repo")

import numpy as np
import ml_dtypes

import concourse.bass as bass
import concourse.mybir as mybir
import concourse.tile as tile
from concourse.bass_utils import run_bass_kernel_spmd
from concourse.masks import make_identity

B, N, S, C, H = 16, 64, 4096, 384, 6
DH = C // H  # 64
NCORES = 8
BPC = B // NCORES  # batches per core = 2
CT = C // 128  # c-tiles = 3
# chunk boundaries: a tiny first chunk so subtile 0's key data lands right
# after the engines finish booting, then realign to 512-token chunks
CHUNK_BOUNDS = [0, 128] + list(range(640, S, 512)) + [S]
CHUNKS = list(zip(CHUNK_BOUNDS[:-1], CHUNK_BOUNDS[1:]))

F32 = mybir.dt.float32
F32R = mybir.dt.float32r
BF16 = mybir.dt.bfloat16

LAST_RESULT = None  # stash of BassKernelResults for profiling in test.py


def _split_multiwaits(nc):
    """walrus codegen in this toolchain accepts at most one sync-wait per
    instruction; hoist extras onto standalone wait-only EventSemaphore
    instructions placed immediately before (same engine, so ordering holds)."""
    for fn in nc.m.functions:
        for blk in fn.blocks:
            new = []
            for inst in blk.instructions:
                si = inst.sync_info
                if si is not None and si.on_wait and len(si.on_wait) > 1:
                    for w in si.on_wait[:-1]:
                        ev = mybir.InstEventSemaphore(
                            name=nc.get_next_instruction_name(), ins=[], outs=[]
                        )
                        ev.engine = inst.engine
                        ev.sync_info = mybir.SyncInfo(on_wait=[w], on_update=[])
                        new.append(ev)
                    inst.sync_info = mybir.SyncInfo(
                        on_wait=[si.on_wait[-1]], on_update=si.on_update
                    )
                new.append(inst)
            blk.instructions = new


def _build_kernel():
    nc = bass.Bass()
    # keyT in per-chunk partition-major blocks: columns [3*s0, 3*s1) of
    # partition p hold chunk (s0,s1)'s [ct, s] rows for key row ct*128+p
    keyT_d = nc.declare_dram_parameter("keyT", [BPC, 128, CT * S], F32R, isOutput=False)
    tc_d = nc.declare_dram_parameter("tc", [BPC, 128, CT * C], F32R, isOutput=False)
    wvt_d = nc.declare_dram_parameter("wvt", [128, CT * C], F32R, isOutput=False)
    wpt_d = nc.declare_dram_parameter("wpt", [128, CT * C], BF16, isOutput=False)
    out_d = nc.declare_dram_parameter("out", [BPC, N, C], F32, isOutput=True)

    with tile.TileContext(nc) as tc:
        with (
            tc.tile_pool(name="consts", bufs=1) as consts,
            tc.tile_pool(name="perb", bufs=2) as perb,
            tc.tile_pool(name="keyp", bufs=4) as keyp,
            tc.tile_pool(name="work", bufs=4) as work,
            tc.tile_pool(name="epi", bufs=2) as epi,
            tc.tile_pool(name="ps_attn", bufs=3, space="PSUM") as ps_attn,
            tc.tile_pool(name="ps_v", bufs=2, space="PSUM") as ps_v,
            tc.tile_pool(name="ps_o", bufs=1, space="PSUM") as ps_o,
            tc.tile_pool(name="ps_epi", bufs=1, space="PSUM") as ps_epi,
        ):
            # Initial transfers, latency-ordered.  Act's sequencer finishes
            # boot slightly before SP's, and the two trigger into separate
            # queue sets, so interleave: Act carries subtile 0's inputs
            # (kt chunk 0, tc ct 0, then the rest of tc), SP carries wvt and
            # kt chunk 1.  wpt (epilogue-only) goes last.
            s0, s1 = CHUNKS[0]
            kt_c0 = keyp.tile([128, CT, s1 - s0], F32R, tag="kt")
            nc.scalar.dma_start(
                out=kt_c0[:], in_=keyT_d[0][:, CT * s0 : CT * s1]
            )
            tc_first = perb.tile([128, CT, C], F32R, tag="tc_sb")
            nc.scalar.dma_start(out=tc_first[:, 0, :], in_=tc_d[0][:, 0:C])
            nc.scalar.dma_start(
                out=tc_first[:, 1:CT, :], in_=tc_d[0][:, C : CT * C]
            )
            wvt_sb = consts.tile([128, CT, C], F32R)  # [c_in_p, ct, c_out]
            nc.sync.dma_start(out=wvt_sb[:], in_=wvt_d[:])
            s0, s1 = CHUNKS[1]
            kt_c1 = keyp.tile([128, CT, s1 - s0], F32R, tag="kt")
            nc.sync.dma_start(out=kt_c1[:], in_=keyT_d[0][:, CT * s0 : CT * s1])
            wpt_sb = consts.tile([128, CT, C], BF16)  # [hd_p, ct, c_out]
            nc.sync.dma_start(out=wpt_sb[:], in_=wpt_d[:])
            # two stacked 64x64 identities so transposes of partition-offset-64
            # slices have a matching-base-partition rhs
            ident2 = consts.tile([128, N], BF16)
            make_identity(nc, ident2[0:N, :])
            make_identity(nc, ident2[N : 2 * N, :])

            for b in range(BPC):
                if b == 0:
                    tc_sb = tc_first
                else:
                    tc_sb = perb.tile([128, CT, C], F32R, tag="tc_sb")
                    nc.sync.dma_start(out=tc_sb[:], in_=tc_d[b][:])
                # per-group accumulator, head-PAIR packed: for pair p, partition
                # rows 0..63 = head 2p groups, rows 64..127 = head 2p+1 groups;
                # col 64 = counts for both heads; cols 0..63 / 65..128 hold the
                # two heads' v-sums (off-diagonal blocks are junk, never read).
                # Zeroed explicitly; the accumulating matmuls use start=False so
                # their order doesn't matter (add-or-overwrite onto zeros commutes).
                o_ps = ps_o.tile([128, CT, 2 * DH + 2], F32)
                nc.vector.memset(o_ps[:], 0.0)

                # software pipeline: issue subtile i's o-matmuls AFTER subtile
                # i+1's attn/v matmuls, so the PE never head-of-line-blocks on
                # DVE's one-hot for the current subtile.
                pending = None  # (aT, v65) of previous subtile

                def flush_o(stop):
                    aT_p, v65_p = pending
                    for p in range(CT):
                        nc.tensor.matmul(
                            o_ps[:, p, :],
                            aT_p[:].rearrange("q h n -> q (h n)")[:, 2 * p * N : (2 * p + 2) * N],
                            v65_p[:].rearrange("q h d -> q (h d)")[
                                :, 2 * p * (DH + 1) : (2 * p + 2) * (DH + 1)
                            ],
                            start=False,
                            stop=stop,
                            skip_group_check=True,
                        )

                for ci, (s0, s1) in enumerate(CHUNKS):
                    if b == 0 and ci == 0:
                        kt_sb = kt_c0
                    elif b == 0 and ci == 1:
                        kt_sb = kt_c1
                    else:
                        kt_sb = keyp.tile([128, CT, s1 - s0], F32R, tag="kt")
                        nc.sync.dma_start(
                            out=kt_sb[:], in_=keyT_d[b][:, CT * s0 : CT * s1]
                        )
                    for sub in range((s1 - s0) // 128):
                        sl = slice(sub * 128, (sub + 1) * 128)
                        attn_ps = ps_attn.tile([128, C], F32)
                        v_ps = ps_v.tile([128, C], F32)
                        # all attn matmuls first so the logit group closes
                        # ~3 matmuls earlier and DVE's argmax starts sooner
                        for ct in range(CT):
                            nc.tensor.matmul(
                                attn_ps[:],
                                kt_sb[:, ct, sl],
                                tc_sb[:, ct, :],
                                start=(ct == 0),
                                stop=(ct == CT - 1),
                            )
                        for ct in range(CT):
                            nc.tensor.matmul(
                                v_ps[:],
                                kt_sb[:, ct, sl],
                                wvt_sb[:, ct, :],
                                start=(ct == 0),
                                stop=(ct == CT - 1),
                            )
                        if pending is not None:
                            flush_o(stop=False)
                        # per-head argmax -> one-hot (bf16); both ops read
                        # PSUM so they must stay on DVE (GpSimd/Pool cannot
                        # access PSUM)
                        gmax = work.tile([128, H], F32)
                        nc.vector.reduce_max(
                            out=gmax[:],
                            in_=attn_ps[:].rearrange("p (h n) -> p h n", h=H),
                            axis=mybir.AxisListType.X,
                        )
                        aT = work.tile([128, H, N], BF16)
                        g = gmax[:]
                        g_bcast = bass.AP(
                            tensor=g.tensor, offset=g.offset,
                            ap=[g.ap[0], g.ap[1], [0, N]],
                        )
                        nc.vector.tensor_tensor(
                            out=aT[:],
                            in0=attn_ps[:].rearrange("p (h n) -> p h n", h=H),
                            in1=g_bcast,
                            op=mybir.AluOpType.is_equal,
                        )
                        # v (bf16) with a ones-column per head for counts
                        v65 = work.tile([128, H, DH + 1], BF16)
                        nc.scalar.copy(
                            out=v65[:, :, 0:DH],
                            in_=v_ps[:].rearrange("p (h d) -> p h d", h=H),
                        )
                        nc.gpsimd.memset(v65[:, :, DH : DH + 1], 1.0)
                        pending = (aT, v65)
                flush_o(stop=True)
                pending = None
                # epilogue for this b, pipelined per head-pair: scale by
                # 1/(cnt+1) (cnt in col 64 of pair p for both heads) straight
                # out of PSUM, transpose to [hd, n], copy, and accumulate the
                # output projection.  Each pair's DVE work overlaps the
                # previous pair's PE/Act work, shortening the serial tail.
                scl = epi.tile([128, CT], F32)
                osc = epi.tile([128, CT, DH], BF16)
                oT_ps = ps_epi.tile([128, CT, N], BF16)
                oT_sb = epi.tile([128, CT, N], BF16)
                out_ps = ps_epi.tile([N, C], F32)
                for p in range(CT):
                    nc.vector.tensor_scalar(
                        out=scl[:, p : p + 1],
                        in0=o_ps[:, p, DH : DH + 1],
                        scalar1=1.0,
                        scalar2=None,
                        op0=mybir.AluOpType.add,
                    )
                    nc.vector.reciprocal(
                        out=scl[:, p : p + 1], in_=scl[:, p : p + 1]
                    )
                    nc.vector.tensor_scalar(
                        out=osc[0:N, p, :],
                        in0=o_ps[0:N, p, 0:DH],
                        scalar1=scl[0:N, p : p + 1],
                        scalar2=None,
                        op0=mybir.AluOpType.mult,
                    )
                    nc.vector.tensor_scalar(
                        out=osc[N : 2 * N, p, :],
                        in0=o_ps[N : 2 * N, p, DH + 1 : 2 * DH + 1],
                        scalar1=scl[N : 2 * N, p : p + 1],
                        scalar2=None,
                        op0=mybir.AluOpType.mult,
                    )
                    # osc[0:64, p, :] = [n, dh] of head 2p -> oT rows 128p+dh;
                    # osc[64:128, p, :] -> oT rows 128p+64+dh
                    nc.tensor.transpose(
                        oT_ps[0:N, p, :], osc[0:N, p, :], ident2[0:N, :]
                    )
                    nc.tensor.transpose(
                        oT_ps[N : 2 * N, p, :],
                        osc[N : 2 * N, p, :],
                        ident2[N : 2 * N, :],
                    )
                    nc.scalar.copy(out=oT_sb[:, p, :], in_=oT_ps[:, p, :])
                    nc.tensor.matmul(
                        out_ps[:],
                        oT_sb[:, p, :],
                        wpt_sb[:, p, :],
                        start=(p == 0),
                        stop=(p == CT - 1),
                    )
                out_sb = epi.tile([N, C], F32)
                nc.scalar.copy(out=out_sb[:], in_=out_ps[:])
                nc.sync.dma_start(out=out_d[b], in_=out_sb[:])

    _split_multiwaits(nc)
    return nc


_NC_CACHE = None


def _get_nc():
    global _NC_CACHE
    if _NC_CACHE is None:
        _NC_CACHE = _build_kernel()
    return _NC_CACHE


def _perm_blocks(a, bounds):
    """a: [C, X] -> [128, sum over chunks of CT*len] with per-chunk
    partition-major contiguous blocks."""
    a3 = a.reshape(CT, 128, -1)  # [ct, p, X]
    parts = []
    for s0, s1 in zip(bounds[:-1], bounds[1:]):
        blk = a3[:, :, s0:s1].transpose(1, 0, 2).reshape(128, -1)
        parts.append(blk)
    return np.ascontiguousarray(np.concatenate(parts, axis=1))


def kernel(query, key, Wq, Wk, Wv, Wp, bp):
    global LAST_RESULT
    query = np.ascontiguousarray(query, dtype=np.float32)
    key = np.ascontiguousarray(key, dtype=np.float32)
    Wq = np.asarray(Wq, dtype=np.float32)
    Wk = np.asarray(Wk, dtype=np.float32)
    Wv = np.asarray(Wv, dtype=np.float32)
    Wp = np.asarray(Wp, dtype=np.float32)
    bp = np.asarray(bp, dtype=np.float32)

    # host prep: t[b,h,n,:] = Wk_h^T Wq_h query[b,n]  (tiny; never touches `key`)
    q = query @ Wq.T  # [B, N, C]
    qh = q.reshape(B, N, H, DH).transpose(0, 2, 1, 3)  # [B,H,N,DH]
    Wk_h = Wk.reshape(H, DH, C)
    t = np.einsum("bhnd,hdc->bhnc", qh, Wk_h)  # [B,H,N,C]
    # Tc[b]: [C, (h n)] with column h*N+n = t[b,h,n,:], then partition-major
    Tc = t.transpose(0, 3, 1, 2).reshape(B, C, H * N).astype(np.float32)
    keyT = key.transpose(0, 2, 1).astype(np.float32)  # [B, C, S]

    keyT_pm = np.stack([_perm_blocks(keyT[b], CHUNK_BOUNDS) for b in range(B)])
    Tc_pm = np.stack([_perm_blocks(Tc[b], [0, C]) for b in range(B)])
    WvT_pm = _perm_blocks(Wv.T.astype(np.float32), [0, C])
    WpT_pm = _perm_blocks(Wp.T.astype(np.float32), [0, C]).astype(
        ml_dtypes.bfloat16
    )

    nc = _get_nc()
    in_maps = [
        {
            "keyT": keyT_pm[i * BPC : (i + 1) * BPC],
            "tc": Tc_pm[i * BPC : (i + 1) * BPC],
            "wvt": WvT_pm,
            "wpt": WpT_pm,
        }
        for i in range(NCORES)
    ]
    try:
        res = run_bass_kernel_spmd(nc, in_maps, core_ids=list(range(NCORES)))
    except Exception:
        # transient NRT device errors have been observed; retry once
        res = run_bass_kernel_spmd(nc, in_maps, core_ids=list(range(NCORES)))
    LAST_RESULT = res
    out = np.concatenate([res.results[i]["out"] for i in range(NCORES)], axis=0)
    return (out + bp).astype(np.float32)


# revision 14
# speedup vs baseline: 1.1169x; 1.1169x over previous
"""Trainium2 Bass kernel for nn_AssignAttention (hard-assignment MoE-routing attention).

Math (forward): for each (b, h, key-token s), the key token is hard-assigned to
group n* = argmax_n (q_bhn . k_bhs); output per group = sum of assigned v vectors
scaled by 1/(count+1), then projected.  The straight-through softmax terms cancel
in forward up to ~1e-7, so only the argmax routing matters.

Strategy:
 - Pure data-parallel over batch B=16 across 8 cores (2 batches/core), no collectives.
 - Host precomputes t[b,h,n,:] = Wk_h^T Wq_h query[b,n] so attention logits are
   attn[s, (h,n)] = key[b,s,:] . t[b,h,n,:]  -- one C-contraction against raw key.
 - Host lays out every device tensor partition-major ([128, ct, ...] with each
   partition's data contiguous in HBM): fewer, larger DMA descriptors.
 - Attention logits use float32r matmuls (1 cyc/row, ~13-bit mantissa): measured
   argmax flip-induced error ~0.008 rel, well within tolerance. v/output paths in
   float32r/bf16.
 - Per 128-row s-subtile: argmax over each head's 64 logit columns (free-axis
   reduce_max + one broadcast is_equal -> bf16 one-hot on DVE), then
   PSUM-accumulate head-PAIR-packed o += aT_pair^T @ [v|1|v|1] (the ones column
   yields per-group counts).  The o-matmuls are flushed in one burst per CHUNK
   (after the next chunk's first subtile's attn/v), not per subtile: the PE
   pays its f32r<->bf16 reconfiguration penalty (~55ns) twice per burst instead
   of twice per subtile, and the extra pipeline depth gives DVE more slack.
 - Startup: the NEFF boot blocks all sequencers ~7.5us and each DMA trigger
   costs ~0.7us of sequencer time, so the kernel merges kt-chunk-0 plus the
   whole tc tensor of batch 0 into ONE host-prepared transfer (one trigger,
   one completion), uses two tiny 128-token chunks before ramping to 512, and
   runs warmup matmuls to absorb the PE pstate ramp while that first transfer
   lands.  Epilogue scales by 1/(cnt+1) straight out of PSUM with two stride-0
   broadcast multiplies, transposes via PE, projects, DMAs out.
"""
import sys

sys.path.insert(0, "/opt/trn_rl_repo")

import numpy as np
import ml_dtypes

import concourse.bass as bass
import concourse.mybir as mybir
import concourse.tile as tile
from concourse.bass_utils import run_bass_kernel_spmd
from concourse.masks import make_identity

B, N, S, C, H = 16, 64, 4096, 384, 6
DH = C // H  # 64
NCORES = 8
BPC = B // NCORES  # batches per core = 2
CT = C // 128  # c-tiles = 3
# chunk boundaries: two tiny chunks and a half chunk so the DMA pipeline can
# feed the PE as soon as the merged first transfer lands, then 512-token chunks
CHUNK_BOUNDS = [0, 128, 256, 512] + list(range(1024, S, 512)) + [S]
CHUNKS = list(zip(CHUNK_BOUNDS[:-1], CHUNK_BOUNDS[1:]))

F32 = mybir.dt.float32
F32R = mybir.dt.float32r
BF16 = mybir.dt.bfloat16

LAST_RESULT = None  # stash of BassKernelResults for profiling in test.py


def _split_multiwaits(nc):
    """walrus codegen in this toolchain accepts at most one sync-wait per
    instruction; hoist extras onto standalone wait-only EventSemaphore
    instructions placed immediately before (same engine, so ordering holds)."""
    for fn in nc.m.functions:
        for blk in fn.blocks:
            new = []
            for inst in blk.instructions:
                si = inst.sync_info
                if si is not None and si.on_wait and len(si.on_wait) > 1:
                    for w in si.on_wait[:-1]:
                        ev = mybir.InstEventSemaphore(
                            name=nc.get_next_instruction_name(), ins=[], outs=[]
                        )
                        ev.engine = inst.engine
                        ev.sync_info = mybir.SyncInfo(on_wait=[w], on_update=[])
                        new.append(ev)
                    inst.sync_info = mybir.SyncInfo(
                        on_wait=[si.on_wait[-1]], on_update=si.on_update
                    )
                new.append(inst)
            blk.instructions = new


def _build_kernel():
    nc = bass.Bass()
    # pre: merged [kt chunk0 | tc] for batch 0, partition-major per ct:
    # per (p, ct): [kt(128 tokens) | tc(C columns)]
    pre_d = nc.declare_dram_parameter(
        "pre", [128, CT * (128 + C)], F32R, isOutput=False
    )
    # keyT in per-chunk partition-major blocks; batch 0's first chunk lives in
    # pre_d instead (columns [0, CT*128) here are unused for b=0)
    keyT_d = nc.declare_dram_parameter("keyT", [BPC, 128, CT * S], F32R, isOutput=False)
    tc_d = nc.declare_dram_parameter("tc", [BPC, 128, CT * C], F32R, isOutput=False)
    wvt_d = nc.declare_dram_parameter("wvt", [128, CT * C], F32R, isOutput=False)
    wpt_d = nc.declare_dram_parameter("wpt", [128, CT * C], BF16, isOutput=False)
    out_d = nc.declare_dram_parameter("out", [BPC, N, C], F32, isOutput=True)

    with tile.TileContext(nc) as tc:
        with (
            tc.tile_pool(name="consts", bufs=1) as consts,
            tc.tile_pool(name="perb", bufs=2) as perb,
            tc.tile_pool(name="keyp", bufs=6) as keyp,
            tc.tile_pool(name="work", bufs=8) as work,
            tc.tile_pool(name="epi", bufs=2) as epi,
            tc.tile_pool(name="ps_attn", bufs=3, space="PSUM") as ps_attn,
            tc.tile_pool(name="ps_v", bufs=2, space="PSUM") as ps_v,
            tc.tile_pool(name="ps_o", bufs=1, space="PSUM") as ps_o,
            tc.tile_pool(name="ps_epi", bufs=1, space="PSUM") as ps_epi,
        ):
            # one merged transfer delivers everything subtile 0 needs
            pre_sb = consts.tile([128, CT, 128 + C], F32R)
            nc.sync.dma_start(out=pre_sb[:], in_=pre_d[:])
            kt_c0 = pre_sb[:, :, 0:128]
            tc_b0 = pre_sb[:, :, 128 : 128 + C]
            wvt_sb = consts.tile([128, CT, C], F32R)  # [c_in_p, ct, c_out]
            nc.sync.dma_start(out=wvt_sb[:], in_=wvt_d[:])
            s0, s1 = CHUNKS[1]
            kt_c1 = keyp.tile([128, CT, s1 - s0], F32R, tag="kt")
            nc.sync.dma_start(out=kt_c1[:], in_=keyT_d[0][:, CT * s0 : CT * s1])
            s0, s1 = CHUNKS[2]
            kt_c2 = keyp.tile([128, CT, s1 - s0], F32R, tag="kt")
            nc.sync.dma_start(out=kt_c2[:], in_=keyT_d[0][:, CT * s0 : CT * s1])
            wpt_sb = consts.tile([128, CT, C], BF16)  # [hd_p, ct, c_out]
            nc.sync.dma_start(out=wpt_sb[:], in_=wpt_d[:])
            # two stacked 64x64 identities so transposes of partition-offset-64
            # slices have a matching-base-partition rhs
            ident2 = consts.tile([128, N], BF16)
            make_identity(nc, ident2[0:N, :])
            make_identity(nc, ident2[N : 2 * N, :])

            # PE warmup: back-to-back matmuls on scratch while the first
            # transfer lands, so the pstate ramp completes before real work.
            # The psum bank is never read; its reuse starts with start=True.
            warm_sb = consts.tile([128, 640], BF16)
            nc.gpsimd.memset(warm_sb[:], 0.0)
            warm_ps = ps_attn.tile([128, 512], F32, tag="attn_ps")
            for _ in range(8):
                nc.tensor.matmul(
                    warm_ps[:], warm_sb[:, 0:128], warm_sb[:, 128:640],
                    start=True, stop=True,
                )

            for b in range(BPC):
                if b == 0:
                    tc_sb = tc_b0
                else:
                    tc_t = perb.tile([128, CT, C], F32R, tag="tc_sb")
                    nc.sync.dma_start(out=tc_t[:], in_=tc_d[b][:])
                    tc_sb = tc_t[:, :, :]
                # per-group accumulator, head-PAIR packed: for pair p, partition
                # rows 0..63 = head 2p groups, rows 64..127 = head 2p+1 groups;
                # col 64 = counts for both heads; cols 0..63 / 65..128 hold the
                # two heads' v-sums (off-diagonal blocks are junk, never read).
                # Zeroed explicitly; the accumulating matmuls use start=False so
                # their order doesn't matter (add-or-overwrite onto zeros commutes).
                o_ps = ps_o.tile([128, CT, 2 * DH + 2], F32)
                nc.vector.memset(o_ps[:], 0.0)

                # o-matmuls are flushed one chunk at a time, after the NEXT
                # chunk's first subtile's attn/v, so the PE never waits on
                # DVE's one-hot and pays the bf16<->f32r switch only once per
                # chunk in each direction.
                pending = []  # [(aT, v65), ...] of the previous chunk

                def flush_o(stop):
                    for i, (aT_p, v65_p) in enumerate(pending):
                        last_sub = i == len(pending) - 1
                        for p in range(CT):
                            nc.tensor.matmul(
                                o_ps[:, p, :],
                                aT_p[:].rearrange("q h n -> q (h n)")[
                                    :, 2 * p * N : (2 * p + 2) * N
                                ],
                                v65_p[:].rearrange("q h d -> q (h d)")[
                                    :, 2 * p * (DH + 1) : (2 * p + 2) * (DH + 1)
                                ],
                                start=False,
                                stop=stop and last_sub and p == CT - 1,
                                skip_group_check=True,
                            )
                    pending.clear()

                for ci, (s0, s1) in enumerate(CHUNKS):
                    if b == 0 and ci == 0:
                        kt_sb = kt_c0
                    elif b == 0 and ci == 1:
                        kt_sb = kt_c1[:, :, :]
                    elif b == 0 and ci == 2:
                        kt_sb = kt_c2[:, :, :]
                    else:
                        kt_t = keyp.tile([128, CT, s1 - s0], F32R, tag="kt")
                        nc.sync.dma_start(
                            out=kt_t[:], in_=keyT_d[b][:, CT * s0 : CT * s1]
                        )
                        kt_sb = kt_t[:, :, :]
                    carry = None
                    for sub in range((s1 - s0) // 128):
                        sl = slice(sub * 128, (sub + 1) * 128)
                        attn_ps = ps_attn.tile([128, C], F32)
                        v_ps = ps_v.tile([128, C], F32)
                        # all attn matmuls first so the logit group closes
                        # ~3 matmuls earlier and DVE's argmax starts sooner
                        for ct in range(CT):
                            nc.tensor.matmul(
                                attn_ps[:],
                                kt_sb[:, ct, sl],
                                tc_sb[:, ct, :],
                                start=(ct == 0),
                                stop=(ct == CT - 1),
                            )
                        for ct in range(CT):
                            nc.tensor.matmul(
                                v_ps[:],
                                kt_sb[:, ct, sl],
                                wvt_sb[:, ct, :],
                                start=(ct == 0),
                                stop=(ct == CT - 1),
                            )
                        if sub == 0 and pending:
                            flush_o(stop=False)
                        # per-head argmax -> one-hot (bf16); both ops read
                        # PSUM so they must stay on DVE (GpSimd/Pool cannot
                        # access PSUM)
                        gmax = work.tile([128, H], F32)
                        nc.vector.reduce_max(
                            out=gmax[:],
                            in_=attn_ps[:].rearrange("p (h n) -> p h n", h=H),
                            axis=mybir.AxisListType.X,
                        )
                        aT = work.tile([128, H, N], BF16)
                        g = gmax[:]
                        g_bcast = bass.AP(
                            tensor=g.tensor, offset=g.offset,
                            ap=[g.ap[0], g.ap[1], [0, N]],
                        )
                        nc.vector.tensor_tensor(
                            out=aT[:],
                            in0=attn_ps[:].rearrange("p (h n) -> p h n", h=H),
                            in1=g_bcast,
                            op=mybir.AluOpType.is_equal,
                        )
                        # v (bf16) with a ones-column per head for counts
                        v65 = work.tile([128, H, DH + 1], BF16)
                        nc.scalar.copy(
                            out=v65[:, :, 0:DH],
                            in_=v_ps[:].rearrange("p (h d) -> p h d", h=H),
                        )
                        nc.gpsimd.memset(v65[:, :, DH : DH + 1], 1.0)
                        if carry is None:
                            carry = []
                        carry.append((aT, v65))
                    pending.extend(carry)
                flush_o(stop=True)
                # epilogue for this b: scale by 1/(cnt+1) (cnt in col 64 for
                # both heads of each pair) straight out of PSUM -- two
                # stride-0-broadcast multiplies -- then transpose to [hd, n],
                # project, and DMA out
                scl = epi.tile([128, CT], F32)
                nc.vector.tensor_scalar(
                    out=scl[:],
                    in0=o_ps[:, :, DH],
                    scalar1=1.0,
                    scalar2=None,
                    op0=mybir.AluOpType.add,
                )
                nc.vector.reciprocal(out=scl[:], in_=scl[:])
                osc = epi.tile([128, CT, DH], BF16)
                s0_ = scl[0:N, :]
                s0b = bass.AP(
                    tensor=s0_.tensor, offset=s0_.offset,
                    ap=[s0_.ap[0], s0_.ap[1], [0, DH]],
                )
                nc.vector.tensor_tensor(
                    out=osc[0:N, :, :],
                    in0=o_ps[0:N, :, 0:DH],
                    in1=s0b,
                    op=mybir.AluOpType.mult,
                )
                s1_ = scl[N : 2 * N, :]
                s1b = bass.AP(
                    tensor=s1_.tensor, offset=s1_.offset,
                    ap=[s1_.ap[0], s1_.ap[1], [0, DH]],
                )
                nc.vector.tensor_tensor(
                    out=osc[N : 2 * N, :, :],
                    in0=o_ps[N : 2 * N, :, DH + 1 : 2 * DH + 1],
                    in1=s1b,
                    op=mybir.AluOpType.mult,
                )
                # osc[0:64, p, :] = [n, dh] of head 2p -> oT rows 128p+dh;
                # osc[64:128, p, :] = [n, dh] of head 2p+1 -> oT rows 128p+64+dh
                oT_ps = ps_epi.tile([128, CT, N], BF16)
                for p in range(CT):
                    nc.tensor.transpose(
                        oT_ps[0:N, p, :], osc[0:N, p, :], ident2[0:N, :]
                    )
                    nc.tensor.transpose(
                        oT_ps[N : 2 * N, p, :],
                        osc[N : 2 * N, p, :],
                        ident2[N : 2 * N, :],
                    )
                oT_sb = epi.tile([128, CT, N], BF16)
                nc.scalar.copy(out=oT_sb[:], in_=oT_ps[:])
                out_ps = ps_epi.tile([N, C], F32)
                for ct in range(CT):
                    nc.tensor.matmul(
                        out_ps[:],
                        oT_sb[:, ct, :],
                        wpt_sb[:, ct, :],
                        start=(ct == 0),
                        stop=(ct == CT - 1),
                    )
                out_sb = epi.tile([N, C], F32)
                nc.scalar.copy(out=out_sb[:], in_=out_ps[:])
                nc.sync.dma_start(out=out_d[b], in_=out_sb[:])

    _split_multiwaits(nc)
    return nc


_NC_CACHE = None


def _get_nc():
    global _NC_CACHE
    if _NC_CACHE is None:
        _NC_CACHE = _build_kernel()
    return _NC_CACHE


def _perm_blocks(a, bounds):
    """a: [C, X] -> [128, sum over chunks of CT*len] with per-chunk
    partition-major contiguous blocks (partition p's columns hold rows
    {ct*128+p} of each column-chunk)."""
    a3 = a.reshape(CT, 128, -1)  # [ct, p, X]
    parts = []
    for s0, s1 in zip(bounds[:-1], bounds[1:]):
        blk = a3[:, :, s0:s1].transpose(1, 0, 2).reshape(128, -1)
        parts.append(blk)
    return np.ascontiguousarray(np.concatenate(parts, axis=1))


def kernel(query, key, Wq, Wk, Wv, Wp, bp):
    global LAST_RESULT
    query = np.ascontiguousarray(query, dtype=np.float32)
    key = np.ascontiguousarray(key, dtype=np.float32)
    Wq = np.asarray(Wq, dtype=np.float32)
    Wk = np.asarray(Wk, dtype=np.float32)
    Wv = np.asarray(Wv, dtype=np.float32)
    Wp = np.asarray(Wp, dtype=np.float32)
    bp = np.asarray(bp, dtype=np.float32)

    # host prep: t[b,h,n,:] = Wk_h^T Wq_h query[b,n]  (tiny; never touches `key`)
    q = query @ Wq.T  # [B, N, C]
    qh = q.reshape(B, N, H, DH).transpose(0, 2, 1, 3)  # [B,H,N,DH]
    Wk_h = Wk.reshape(H, DH, C)
    t = np.einsum("bhnd,hdc->bhnc", qh, Wk_h)  # [B,H,N,C]
    # Tc[b]: [C, (h n)] with column h*N+n = t[b,h,n,:], then partition-major
    Tc = t.transpose(0, 3, 1, 2).reshape(B, C, H * N).astype(np.float32)
    keyT = key.transpose(0, 2, 1).astype(np.float32)  # [B, C, S]

    keyT_pm = np.stack([_perm_blocks(keyT[b], CHUNK_BOUNDS) for b in range(B)])
    Tc_pm = np.stack([_perm_blocks(Tc[b], [0, C]) for b in range(B)])
    WvT_pm = _perm_blocks(Wv.T.astype(np.float32), [0, C])
    WpT_pm = _perm_blocks(Wp.T.astype(np.float32), [0, C]).astype(
        ml_dtypes.bfloat16
    )
    # merged first transfer per core (batch 0 of that core): per (p, ct):
    # [kt chunk0 (128) | tc (C)]
    pre_all = []
    for i in range(NCORES):
        b0 = i * BPC
        kt0 = keyT_pm[b0][:, 0 : CT * 128].reshape(128, CT, 128)
        tcb = Tc_pm[b0].reshape(128, CT, C)
        pre_all.append(
            np.ascontiguousarray(
                np.concatenate([kt0, tcb], axis=2).reshape(128, -1)
            )
        )

    nc = _get_nc()
    in_maps = [
        {
            "pre": pre_all[i],
            "keyT": keyT_pm[i * BPC : (i + 1) * BPC],
            "tc": Tc_pm[i * BPC : (i + 1) * BPC],
            "wvt": WvT_pm,
            "wpt": WpT_pm,
        }
        for i in range(NCORES)
    ]
    try:
        res = run_bass_kernel_spmd(nc, in_maps, core_ids=list(range(NCORES)))
    except Exception:
        # transient NRT device errors have been observed; retry once
        res = run_bass_kernel_spmd(nc, in_maps, core_ids=list(range(NCORES)))
    LAST_RESULT = res
    out = np.concatenate([res.results[i]["out"] for i in range(NCORES)], axis=0)
    return (out + bp).astype(np.float32)
